# revision 8
# baseline (speedup 1.0000x reference)
"""Trainium2 Bass kernel for a multi-head attention block.

Problem: y = AttnBlock(x) with B=4, T=2048, D=1024, H=16 heads, head_dim=64.
    q = x@Wq.T+bq, k = x@Wk.T+bk, v = x@Wv.T+bv  (per-head reshape)
    y = softmax(q k^T / 8) v, concat heads, y@Wo.T+bo

Sharding over 8 cores: data-parallel over B (4) x tensor-parallel over head
groups (2 groups of 8 heads).  Each core computes its batch's attention for
its 8 heads plus the partial output projection over its 512 columns of Wo;
the two partials per batch are summed on the host (plus exact host-side
bias corrections for bv/bo).

Device dataflow (per core, all matmuls on TensorE as float32r except the
attention-weight matmul which is bf16):
    phase 1: qT,kT [512,2048] = WqT.T @ xT   (e on partitions -> per-
             partition bias add on DVE), v [2048,520-packed] bf16
    phase 2: per (q-chunk 512, head-pair): S^T chunks [128k,512q] via
             row-tiled K=64 matmul pairs; exp on ScalarE (scale=1/8,
             no max-subtraction: scores ~ N(0,1), exp is safe) -> bf16;
             AV matmul with a fused ones-column producing unnormalized
             O^T [64,512] + sumexp row; normalize via DVE reciprocal +
             DMA partition-broadcast + DVE multiply.
    phase 3: y[t,e] = O^T.T @ WoT accumulated over all 8 heads.
"""

import math

import numpy as np

B, T, D = 4, 2048, 1024
H, HD = 16, 64
EL = 512  # local (per-core) e-width: 8 heads * 64
N_CORES = 8
NPAIR = 4  # head pairs per core
QC = 4  # q chunks of 512
KC = 16  # k chunks of 128
DC = 8  # d chunks of 128
VW = 65  # v columns per head incl. ones column


def build_kernel():
    import concourse.bass as bass
    import concourse.mybir as mybir
    import concourse.tile as tile
    from concourse import bacc

    f32 = mybir.dt.float32
    f32r = mybir.dt.float32r
    bf16 = mybir.dt.bfloat16
    Exp = mybir.ActivationFunctionType.Exp
    Copy = mybir.ActivationFunctionType.Copy

    nc = bacc.Bacc("TRN2", target_bir_lowering=False, debug=False,
                   num_devices=N_CORES)

    xT = nc.dram_tensor("xT", [D, T], f32, kind="ExternalInput").ap()
    wqT = nc.dram_tensor("wqT", [D, EL], f32, kind="ExternalInput").ap()
    wkT = nc.dram_tensor("wkT", [D, EL], f32, kind="ExternalInput").ap()
    wvT = nc.dram_tensor("wvT", [D, EL], f32, kind="ExternalInput").ap()
    woT = nc.dram_tensor("woT", [EL, D], f32, kind="ExternalInput").ap()
    bq_d = nc.dram_tensor("bq", [EL], f32, kind="ExternalInput").ap()
    bk_d = nc.dram_tensor("bk", [EL], f32, kind="ExternalInput").ap()
    y_d = nc.dram_tensor("y", [T, D], f32, kind="ExternalOutput").ap()

    with tile.TileContext(nc) as tc:
        with (
            tc.tile_pool(name="p_const", bufs=1) as p_const,
            tc.tile_pool(name="p_qk", bufs=1) as p_qk,
            tc.tile_pool(name="p_v", bufs=1) as p_v,
            tc.tile_pool(name="p_wo", bufs=1) as p_wo,
            tc.tile_pool(name="p_ps_av", bufs=1, space="PSUM") as p_ps_av,
        ):
            # ---- constants: bias chunks [128,1] ----
            bq_t = []
            bk_t = []
            for ec in range(4):
                bqc = p_const.tile([128, 1], f32, name=f"bq{ec}")
                bkc = p_const.tile([128, 1], f32, name=f"bk{ec}")
                nc.sync.dma_start(
                    bqc[:], bq_d[bass.ds(ec * 128, 128)].rearrange(
                        "(p o) -> p o", o=1))
                nc.sync.dma_start(
                    bkc[:], bk_d[bass.ds(ec * 128, 128)].rearrange(
                        "(p o) -> p o", o=1))
                bq_t.append(bqc)
                bk_t.append(bkc)

            # ---- output-proj weights (used late; DMA early, they are small)
            wo_t = []
            for cc in range(4):
                w = p_wo.tile([128, D], f32r, name=f"wo{cc}")
                nc.sync.dma_start(
                    w[:], woT[bass.ds(cc * 128, 128), :].bitcast(f32r))
                wo_t.append(w)

            # ---- qT/kT destination tiles [128, T] (one per head pair) ----
            qT_t = [p_qk.tile([128, T], f32r, name=f"qT{p}") for p in range(NPAIR)]
            kT_t = [p_qk.tile([128, T], f32r, name=f"kT{p}") for p in range(NPAIR)]
            # ---- v tiles: [128, 8*65] bf16 per k-chunk; per head: 64 v cols
            # then a ones column ----
            v_t = [p_v.tile([128, H // 2 * VW], bf16, name=f"v{kc}")
                   for kc in range(KC)]
            for kc in range(KC):
                nc.vector.memset(
                    v_t[kc][:].rearrange("p (h c) -> p h c", c=VW)[:, :, 64:65],
                    1.0)

            # ================= phase 1: projections =================
            with (
                tc.tile_pool(name="p_w", bufs=1) as p_w,
                tc.tile_pool(name="p_x", bufs=1) as p_x,
            ):
                wq_t, wk_t, wv_t = [], [], []
                for dc in range(DC):
                    for lst, src, nm in ((wq_t, wqT, "q"), (wk_t, wkT, "k"),
                                         (wv_t, wvT, "v")):
                        w = p_w.tile([128, EL], f32r, name=f"w{nm}{dc}")
                        nc.sync.dma_start(
                            w[:], src[bass.ds(dc * 128, 128), :].bitcast(f32r))
                        lst.append(w)

                for tc4 in range(4):  # t-chunks of 512
                    tsl = bass.ds(tc4 * 512, 512)
                    x_t = []
                    for dc in range(DC):
                        xx = p_x.tile([128, 512], f32r, name=f"x{tc4}_{dc}", tag="x", bufs=12)
                        nc.sync.dma_start(
                            xx[:],
                            xT[bass.ds(dc * 128, 128), tsl].bitcast(f32r))
                        x_t.append(xx)
                    # qT / kT: out [e 128, t 512]
                    for w_list, dst, b_list in ((wq_t, qT_t, bq_t),
                                                (wk_t, kT_t, bk_t)):
                        for ec in range(4):
                            ps = p_ps_av.tile([128, 512], f32,
                                              name=f"psp{tc4}_{ec}", tag="av", bufs=4)
                            for dc in range(DC):
                                nc.tensor.matmul(
                                    ps[:],
                                    w_list[dc][:, bass.ds(ec * 128, 128)],
                                    x_t[dc][:],
                                    start=(dc == 0), stop=(dc == DC - 1))
                            nc.vector.tensor_scalar_add(
                                dst[ec][:, tsl], ps[:], b_list[ec][:])
                    # v: out [t 128, e 512] -> packed bf16 with ones cols
                    for ts in range(4):
                        kc = tc4 * 4 + ts
                        ps = p_ps_av.tile([128, 512], f32, name=f"psv{kc}", tag="av", bufs=4)
                        for dc in range(DC):
                            nc.tensor.matmul(
                                ps[:],
                                x_t[dc][:, bass.ds(ts * 128, 128)],
                                wv_t[dc][:],
                                start=(dc == 0), stop=(dc == DC - 1))
                        nc.scalar.activation(
                            v_t[kc][:].rearrange(
                                "p (h c) -> p h c", c=VW)[:, :, 0:64],
                            ps[:].rearrange("p (h c) -> p h c", c=64),
                            Copy)

            # ================= phase 2: attention =================
            # Emission order is tuned to keep the in-order PE stream dense:
            # per (qc, pair) we alternate score-chunk matmuls with the AV
            # matmuls of the previous chunk, and sprinkle the previous
            # q-chunk's output-projection groups between pairs.
            with (
                tc.tile_pool(name="p_es", bufs=1) as p_es,
                tc.tile_pool(name="p_ot", bufs=1) as p_ot,
                tc.tile_pool(name="p_bc", bufs=1) as p_bc,
                tc.tile_pool(name="p_y", bufs=1) as p_y,
                tc.tile_pool(name="p_ps_sc", bufs=1, space="PSUM") as p_ps_sc,
            ):
                oT_t = [p_ot.tile([128, T], f32r, name=f"oT{p}")
                        for p in range(NPAIR)]

                def emit_outproj_group(qc, ts, ecc):
                    t128 = bass.ds(qc * 512 + ts * 128, 128)
                    psy = p_ps_av.tile([128, 512], f32,
                                       name=f"psy{qc}_{ts}_{ecc}",
                                       tag="av", bufs=4)
                    for cc in range(4):
                        nc.tensor.matmul(
                            psy[:],
                            oT_t[cc][:, t128],
                            wo_t[cc][:, bass.ds(ecc * 512, 512)],
                            start=(cc == 0), stop=(cc == 3))
                    yt = p_y.tile([128, 512], f32,
                                  name=f"y{qc}_{ts}_{ecc}", tag="y", bufs=4)
                    nc.vector.tensor_copy(yt[:], psy[:])
                    nc.sync.dma_start(
                        y_d[t128, bass.ds(ecc * 512, 512)], yt[:])

                def emit_av(pair, kc2, es, psA, psB):
                    hA, hB = 2 * pair, 2 * pair + 1
                    for j in range(2):
                        kc = kc2 * 2 + j
                        off = j * 1024
                        nc.tensor.matmul(
                            psA[0:VW, :],
                            v_t[kc][:, bass.ds(hA * VW, VW)],
                            es[:, bass.ds(off, 512)],
                            start=(kc == 0), stop=(kc == KC - 1))
                        nc.tensor.matmul(
                            psB[0:VW, :],
                            v_t[kc][:, bass.ds(hB * VW, VW)],
                            es[:, bass.ds(off + 512, 512)],
                            start=(kc == 0), stop=(kc == KC - 1))

                for qc in range(QC):
                    qsl = bass.ds(qc * 512, 512)
                    for pair in range(NPAIR):
                        psA = p_ps_av.tile([128, 512], f32,
                                           name=f"psA{qc}_{pair}", tag="av",
                                           bufs=4)
                        psB = p_ps_av.tile([128, 512], f32,
                                           name=f"psB{qc}_{pair}", tag="av",
                                           bufs=4)
                        es_t = []
                        for kc2 in range(KC // 2):
                            sc = p_ps_sc.tile([128, 2048], f32,
                                              name=f"sc{qc}_{pair}_{kc2}",
                                              tag="sc", bufs=1)
                            es = p_es.tile([128, 2048], bf16,
                                           name=f"es{qc}_{pair}_{kc2}",
                                           tag="es", bufs=9)
                            for j in range(2):
                                kc = kc2 * 2 + j
                                ksl = bass.ds(kc * 128, 128)
                                off = j * 1024
                                nc.tensor.matmul(
                                    sc[:, bass.ds(off, 512)],
                                    kT_t[pair][0:64, ksl],
                                    qT_t[pair][0:64, qsl],
                                    start=True, stop=True)
                                nc.tensor.matmul(
                                    sc[:, bass.ds(off + 512, 512)],
                                    kT_t[pair][64:128, ksl],
                                    qT_t[pair][64:128, qsl],
                                    start=True, stop=True)
                            nc.scalar.activation(
                                es[:], sc[:], Exp, scale=1.0 / math.sqrt(HD))
                            es_t.append(es)
                            if kc2 >= 1:
                                emit_av(pair, kc2 - 1, es_t[kc2 - 1], psA, psB)
                            if qc > 0 and kc2 == 3:
                                emit_outproj_group(qc - 1, pair, 0)
                            if qc > 0 and kc2 == 7:
                                emit_outproj_group(qc - 1, pair, 1)
                        emit_av(pair, 7, es_t[7], psA, psB)
                        # --- normalize ---
                        # sumexp lives at psum partition 64 (the ones column
                        # of v_aug); reciprocal must stay on that partition,
                        # then DMA partition-broadcasts it down to 0:64.
                        r = p_bc.tile([128, 1024], f32, name=f"r{qc}_{pair}",
                                      tag="r", bufs=2)
                        bcA = p_bc.tile([64, 512], f32,
                                        name=f"bcA{qc}_{pair}", tag="bc",
                                        bufs=4)
                        bcB = p_bc.tile([64, 512], f32,
                                        name=f"bcB{qc}_{pair}", tag="bc",
                                        bufs=4)
                        nc.vector.reciprocal(r[64:65, 0:512], psA[64:65, :])
                        nc.vector.reciprocal(r[64:65, 512:1024],
                                             psB[64:65, :])
                        nc.sync.dma_start(
                            bcA[:],
                            r[64:65, 0:512].rearrange(
                                "p (o f) -> p o f", o=1).to_broadcast(
                                    (1, 64, 512)))
                        nc.sync.dma_start(
                            bcB[:],
                            r[64:65, 512:1024].rearrange(
                                "p (o f) -> p o f", o=1).to_broadcast(
                                    (1, 64, 512)))
                        nc.vector.tensor_mul(
                            oT_t[pair][0:64, qsl], psA[0:64, :], bcA[:])
                        # head B: normalize at partitions 0:64, then DMA-hop
                        # to partitions 64:128 of the O^T tile
                        tmp = p_bc.tile([64, 512], f32r,
                                        name=f"tmp{qc}_{pair}", tag="tmp",
                                        bufs=2)
                        nc.vector.tensor_mul(tmp[:], psB[0:64, :], bcB[:])
                        nc.sync.dma_start(oT_t[pair][64:128, qsl], tmp[:])

                # tail: output projection for the last q-chunk
                for ts in range(4):
                    for ecc in range(2):
                        emit_outproj_group(QC - 1, ts, ecc)

    nc.compile()
    return nc


_NC_CACHE = None
TRACE = False
LAST_EXEC_NS = None
LAST_RESULTS = None


def _get_nc():
    global _NC_CACHE
    if _NC_CACHE is None:
        _NC_CACHE = build_kernel()
    return _NC_CACHE


def kernel(x, Wq, bq, Wk, bk, Wv, bv, Wo, bo):
    from concourse.bass_utils import run_bass_kernel_spmd

    x = np.ascontiguousarray(np.asarray(x, dtype=np.float32))
    Wq = np.asarray(Wq, dtype=np.float32)
    Wk = np.asarray(Wk, dtype=np.float32)
    Wv = np.asarray(Wv, dtype=np.float32)
    Wo = np.asarray(Wo, dtype=np.float32)
    bq = np.asarray(bq, dtype=np.float32)
    bk = np.asarray(bk, dtype=np.float32)
    bv = np.asarray(bv, dtype=np.float32)
    bo = np.asarray(bo, dtype=np.float32)

    in_maps = []
    for core in range(N_CORES):
        b = core // 2
        g = core % 2
        cols = slice(g * EL, (g + 1) * EL)
        in_maps.append({
            "xT": np.ascontiguousarray(x[b].T),
            "wqT": np.ascontiguousarray(Wq[cols, :].T),
            "wkT": np.ascontiguousarray(Wk[cols, :].T),
            "wvT": np.ascontiguousarray(Wv[cols, :].T),
            "woT": np.ascontiguousarray(Wo[:, cols].T),
            "bq": np.ascontiguousarray(bq[cols]),
            "bk": np.ascontiguousarray(bk[cols]),
        })

    nc = _get_nc()
    global LAST_EXEC_NS, LAST_RESULTS
    res = run_bass_kernel_spmd(nc, in_maps, list(range(N_CORES)),
                               trace=TRACE)
    LAST_EXEC_NS = res.exec_time_ns
    LAST_RESULTS = res

    y = np.empty((B, T, D), dtype=np.float32)
    for b in range(B):
        acc = res.results[2 * b]["y"].astype(np.float32) + \
            res.results[2 * b + 1]["y"].astype(np.float32)
        # exact host-side bias corrections: softmax rows sum to 1, so the
        # v-bias contributes Wo[:, cols] @ bv[cols] per group; plus bo.
        acc += bo[None, :]
        acc += (Wo[:, 0:EL] @ bv[0:EL] + Wo[:, EL:2 * EL] @ bv[EL:2 * EL])[None, :]
        y[b] = acc
    return y


# revision 11
# speedup vs baseline: 1.0902x; 1.0902x over previous
"""Trainium2 Bass kernel for a multi-head attention block.

Problem: y = AttnBlock(x) with B=4, T=2048, D=1024, H=16 heads, head_dim=64.
    q = x@Wq.T+bq, k = x@Wk.T+bk, v = x@Wv.T+bv  (per-head reshape)
    y = softmax(q k^T / 8) v, concat heads, y@Wo.T+bo

Sharding over 8 cores: data-parallel over B (4) x tensor-parallel over head
groups (2 groups of 8 heads).  Each core computes its batch's attention for
its 8 heads plus the partial output projection over its 512 columns of Wo;
the two partials per batch are summed on the host (plus exact host-side
bias corrections for bv/bo).

Device dataflow (per core, all matmuls on TensorE as float32r except the
attention-weight matmul which is bf16):
    phase 1: qT,kT [512,2048] = WqT.T @ xT   (e on partitions -> per-
             partition bias add on DVE), v [2048,520-packed] bf16
    phase 2: per (q-chunk 512, head-pair): S^T chunks [128k,512q] via
             row-tiled K=64 matmul pairs; exp on ScalarE (scale=1/8,
             no max-subtraction: scores ~ N(0,1), exp is safe) -> bf16;
             AV matmul with a fused ones-column producing unnormalized
             O^T [64,512] + sumexp row; normalize via DVE reciprocal +
             DMA partition-broadcast + DVE multiply.
    phase 3: y[t,e] = O^T.T @ WoT accumulated over all 8 heads.
"""

import math

import numpy as np

B, T, D = 4, 2048, 1024
H, HD = 16, 64
EL = 512  # local (per-core) e-width: 8 heads * 64
N_CORES = 8
NPAIR = 4  # head pairs per core
QC = 4  # q chunks of 512
KC = 16  # k chunks of 128
DC = 8  # d chunks of 128
VW = 65  # v columns per head incl. ones column


def build_kernel():
    import concourse.bass as bass
    import concourse.mybir as mybir
    import concourse.tile as tile
    from concourse import bacc

    f32 = mybir.dt.float32
    f32r = mybir.dt.float32r
    bf16 = mybir.dt.bfloat16
    Exp = mybir.ActivationFunctionType.Exp
    Copy = mybir.ActivationFunctionType.Copy

    nc = bacc.Bacc("TRN2", target_bir_lowering=False, debug=False,
                   num_devices=N_CORES)

    xT = nc.dram_tensor("xT", [D, T], f32, kind="ExternalInput").ap()
    wqT = nc.dram_tensor("wqT", [D, EL], f32, kind="ExternalInput").ap()
    wkT = nc.dram_tensor("wkT", [D, EL], f32, kind="ExternalInput").ap()
    wvT = nc.dram_tensor("wvT", [D, EL], f32, kind="ExternalInput").ap()
    woT = nc.dram_tensor("woT", [EL, D], f32, kind="ExternalInput").ap()
    bq_d = nc.dram_tensor("bq", [EL], f32, kind="ExternalInput").ap()
    bk_d = nc.dram_tensor("bk", [EL], f32, kind="ExternalInput").ap()
    y_d = nc.dram_tensor("y", [T, D], f32, kind="ExternalOutput").ap()

    with tile.TileContext(nc) as tc:
        with (
            tc.tile_pool(name="p_const", bufs=1) as p_const,
            tc.tile_pool(name="p_qk", bufs=1) as p_qk,
            tc.tile_pool(name="p_v", bufs=1) as p_v,
            tc.tile_pool(name="p_wo", bufs=1) as p_wo,
            tc.tile_pool(name="p_ps_av", bufs=1, space="PSUM") as p_ps_av,
        ):
            # ---- constants: bias chunks [128,1] ----
            bq_t = []
            bk_t = []
            for ec in range(4):
                bqc = p_const.tile([128, 1], f32, name=f"bq{ec}")
                bkc = p_const.tile([128, 1], f32, name=f"bk{ec}")
                nc.sync.dma_start(
                    bqc[:], bq_d[bass.ds(ec * 128, 128)].rearrange(
                        "(p o) -> p o", o=1))
                nc.sync.dma_start(
                    bkc[:], bk_d[bass.ds(ec * 128, 128)].rearrange(
                        "(p o) -> p o", o=1))
                bq_t.append(bqc)
                bk_t.append(bkc)

            # ---- output-proj weights (used late; DMA early, they are small)
            wo_t = []
            for cc in range(4):
                w = p_wo.tile([128, D], f32r, name=f"wo{cc}")
                nc.sync.dma_start(
                    w[:], woT[bass.ds(cc * 128, 128), :].bitcast(f32r))
                wo_t.append(w)

            # ---- qT/kT destination tiles [128, T] (one per head pair) ----
            qT_t = [p_qk.tile([128, T], f32r, name=f"qT{p}") for p in range(NPAIR)]
            kT_t = [p_qk.tile([128, T], f32r, name=f"kT{p}") for p in range(NPAIR)]
            # ---- v tiles: [128, 8*65] bf16 per k-chunk; per head: 64 v cols
            # then a ones column ----
            v_t = [p_v.tile([128, H // 2 * VW], bf16, name=f"v{kc}")
                   for kc in range(KC)]
            for kc in range(KC):
                nc.vector.memset(
                    v_t[kc][:].rearrange("p (h c) -> p h c", c=VW)[:, :, 64:65],
                    1.0)

            # ================= phase 1: projections =================
            with (
                tc.tile_pool(name="p_w", bufs=1) as p_w,
                tc.tile_pool(name="p_x", bufs=1) as p_x,
                tc.tile_pool(name="p_ps_p1", bufs=1, space="PSUM") as p_ps_p1,
            ):
                wq_t, wk_t, wv_t = [], [], []
                for dc in range(DC):
                    for lst, src, nm in ((wq_t, wqT, "q"), (wk_t, wkT, "k"),
                                         (wv_t, wvT, "v")):
                        w = p_w.tile([128, EL], f32r, name=f"w{nm}{dc}")
                        nc.sync.dma_start(
                            w[:], src[bass.ds(dc * 128, 128), :].bitcast(f32r))
                        lst.append(w)

                for tc4 in range(4):  # t-chunks of 512
                    tsl = bass.ds(tc4 * 512, 512)
                    x_t = []
                    for dc in range(DC):
                        xx = p_x.tile([128, 512], f32r, name=f"x{tc4}_{dc}", tag="x", bufs=12)
                        nc.sync.dma_start(
                            xx[:],
                            xT[bass.ds(dc * 128, 128), tsl].bitcast(f32r))
                        x_t.append(xx)
                    # qT / kT: out [e 128, t 512]
                    for w_list, dst, b_list in ((wq_t, qT_t, bq_t),
                                                (wk_t, kT_t, bk_t)):
                        for ec in range(4):
                            ps = p_ps_p1.tile([128, 512], f32,
                                               name=f"psp{tc4}_{ec}",
                                               tag="p1", bufs=4)
                            for dc in range(DC):
                                nc.tensor.matmul(
                                    ps[:],
                                    w_list[dc][:, bass.ds(ec * 128, 128)],
                                    x_t[dc][:],
                                    start=(dc == 0), stop=(dc == DC - 1))
                            nc.vector.tensor_scalar_add(
                                dst[ec][:, tsl], ps[:], b_list[ec][:])
                    # v: out [t 128, e 512] -> packed bf16 with ones cols
                    for ts in range(4):
                        kc = tc4 * 4 + ts
                        ps = p_ps_p1.tile([128, 512], f32, name=f"psv{kc}", tag="p1", bufs=4)
                        for dc in range(DC):
                            nc.tensor.matmul(
                                ps[:],
                                x_t[dc][:, bass.ds(ts * 128, 128)],
                                wv_t[dc][:],
                                start=(dc == 0), stop=(dc == DC - 1))
                        nc.scalar.activation(
                            v_t[kc][:].rearrange(
                                "p (h c) -> p h c", c=VW)[:, :, 0:64],
                            ps[:].rearrange("p (h c) -> p h c", c=64),
                            Copy)

            # ================= phase 2: attention =================
            # Emission order is tuned to keep the in-order PE stream dense:
            # per (qc, pair) we alternate score-chunk matmuls with the AV
            # matmuls of the previous chunk, and sprinkle the previous
            # q-chunk's output-projection groups between pairs.
            with (
                tc.tile_pool(name="p_es", bufs=1) as p_es,
                tc.tile_pool(name="p_ot", bufs=1) as p_ot,
                tc.tile_pool(name="p_bc", bufs=1) as p_bc,
                tc.tile_pool(name="p_y", bufs=1) as p_y,
                tc.tile_pool(name="p_ps_sc", bufs=1, space="PSUM") as p_ps_sc,
            ):
                oT_t = [p_ot.tile([128, T], f32r, name=f"oT{p}")
                        for p in range(NPAIR)]

                def emit_outproj_group(qc, ts, ecc):
                    t128 = bass.ds(qc * 512 + ts * 128, 128)
                    psy = p_ps_av.tile([128, 512], f32,
                                       name=f"psy{qc}_{ts}_{ecc}",
                                       tag="py", bufs=2)
                    for cc in range(4):
                        nc.tensor.matmul(
                            psy[:],
                            oT_t[cc][:, t128],
                            wo_t[cc][:, bass.ds(ecc * 512, 512)],
                            start=(cc == 0), stop=(cc == 3))
                    yt = p_y.tile([128, 512], f32,
                                  name=f"y{qc}_{ts}_{ecc}", tag="y", bufs=4)
                    nc.vector.tensor_copy(yt[:], psy[:])
                    nc.sync.dma_start(
                        y_d[t128, bass.ds(ecc * 512, 512)], yt[:])

                def emit_av(pair, kc2, es, psA, psB):
                    hA, hB = 2 * pair, 2 * pair + 1
                    for j in range(2):
                        kc = kc2 * 2 + j
                        off = j * 1024
                        nc.tensor.matmul(
                            psA[0:VW, :],
                            v_t[kc][:, bass.ds(hA * VW, VW)],
                            es[:, bass.ds(off, 512)],
                            start=(kc == 0), stop=(kc == KC - 1))
                        nc.tensor.matmul(
                            psB[0:VW, :],
                            v_t[kc][:, bass.ds(hB * VW, VW)],
                            es[:, bass.ds(off + 512, 512)],
                            start=(kc == 0), stop=(kc == KC - 1))

                for qc in range(QC):
                    qsl = bass.ds(qc * 512, 512)
                    for pair in range(NPAIR):
                        psA = p_ps_av.tile([128, 512], f32,
                                           name=f"psA{qc}_{pair}", tag="av",
                                           bufs=2)
                        psB = p_ps_av.tile([128, 512], f32,
                                           name=f"psB{qc}_{pair}", tag="av",
                                           bufs=2)
                        es_t = []
                        for kc2 in range(KC // 2):
                            es = p_es.tile([128, 2048], bf16,
                                           name=f"es{qc}_{pair}_{kc2}",
                                           tag="es", bufs=9)
                            for j in range(2):
                                kc = kc2 * 2 + j
                                ksl = bass.ds(kc * 128, 128)
                                sc = p_ps_sc.tile([128, 1024], f32,
                                                  name=f"sc{qc}_{pair}_{kc}",
                                                  tag="sc", bufs=2)
                                nc.tensor.matmul(
                                    sc[:, 0:512],
                                    kT_t[pair][0:64, ksl],
                                    qT_t[pair][0:64, qsl],
                                    start=True, stop=True)
                                nc.tensor.matmul(
                                    sc[:, 512:1024],
                                    kT_t[pair][64:128, ksl],
                                    qT_t[pair][64:128, qsl],
                                    start=True, stop=True)
                                nc.scalar.activation(
                                    es[:, bass.ds(j * 1024, 1024)], sc[:],
                                    Exp, scale=1.0 / math.sqrt(HD))
                            es_t.append(es)
                            if kc2 >= 1:
                                emit_av(pair, kc2 - 1, es_t[kc2 - 1], psA, psB)
                            if qc > 0 and kc2 == 3:
                                emit_outproj_group(qc - 1, pair, 0)
                            if qc > 0 and kc2 == 7:
                                emit_outproj_group(qc - 1, pair, 1)
                        emit_av(pair, 7, es_t[7], psA, psB)
                        # --- normalize ---
                        # Copy the accumulators to SBUF immediately so the
                        # PSUM banks recycle without waiting for the
                        # reciprocal chain; sumexp sits at partition 64 (the
                        # v_aug ones column), reciprocal stays on that partition, then DMA
                        # partition-broadcasts it down to 0:64.
                        stA = p_bc.tile([VW, 512], f32,
                                        name=f"stA{qc}_{pair}", tag="st",
                                        bufs=4)
                        stB = p_bc.tile([VW, 512], f32,
                                        name=f"stB{qc}_{pair}", tag="st",
                                        bufs=4)
                        nc.vector.tensor_copy(stA[:], psA[0:VW, :])
                        nc.vector.tensor_copy(stB[:], psB[0:VW, :])
                        r = p_bc.tile([128, 1024], f32, name=f"r{qc}_{pair}",
                                      tag="r", bufs=2)
                        bcA = p_bc.tile([64, 512], f32,
                                        name=f"bcA{qc}_{pair}", tag="bc",
                                        bufs=4)
                        bcB = p_bc.tile([64, 512], f32,
                                        name=f"bcB{qc}_{pair}", tag="bc",
                                        bufs=4)
                        nc.vector.reciprocal(r[64:65, 0:512], stA[64:65, :])
                        nc.vector.reciprocal(r[64:65, 512:1024],
                                             stB[64:65, :])
                        nc.sync.dma_start(
                            bcA[:],
                            r[64:65, 0:512].rearrange(
                                "p (o f) -> p o f", o=1).to_broadcast(
                                    (1, 64, 512)))
                        nc.sync.dma_start(
                            bcB[:],
                            r[64:65, 512:1024].rearrange(
                                "p (o f) -> p o f", o=1).to_broadcast(
                                    (1, 64, 512)))
                        nc.vector.tensor_mul(
                            oT_t[pair][0:64, qsl], stA[0:64, :], bcA[:])
                        # head B: normalize at partitions 0:64, then DMA-hop
                        # to partitions 64:128 of the O^T tile
                        tmp = p_bc.tile([64, 512], f32r,
                                        name=f"tmp{qc}_{pair}", tag="tmp",
                                        bufs=2)
                        nc.vector.tensor_mul(tmp[:], stB[0:64, :], bcB[:])
                        nc.sync.dma_start(oT_t[pair][64:128, qsl], tmp[:])

                # tail: output projection for the last q-chunk
                for ts in range(4):
                    for ecc in range(2):
                        emit_outproj_group(QC - 1, ts, ecc)

    nc.compile()
    return nc


_NC_CACHE = None
TRACE = False
LAST_EXEC_NS = None
LAST_RESULTS = None


def _get_nc():
    global _NC_CACHE
    if _NC_CACHE is None:
        _NC_CACHE = build_kernel()
    return _NC_CACHE


def kernel(x, Wq, bq, Wk, bk, Wv, bv, Wo, bo):
    from concourse.bass_utils import run_bass_kernel_spmd

    x = np.ascontiguousarray(np.asarray(x, dtype=np.float32))
    Wq = np.asarray(Wq, dtype=np.float32)
    Wk = np.asarray(Wk, dtype=np.float32)
    Wv = np.asarray(Wv, dtype=np.float32)
    Wo = np.asarray(Wo, dtype=np.float32)
    bq = np.asarray(bq, dtype=np.float32)
    bk = np.asarray(bk, dtype=np.float32)
    bv = np.asarray(bv, dtype=np.float32)
    bo = np.asarray(bo, dtype=np.float32)

    in_maps = []
    for core in range(N_CORES):
        b = core // 2
        g = core % 2
        cols = slice(g * EL, (g + 1) * EL)
        in_maps.append({
            "xT": np.ascontiguousarray(x[b].T),
            "wqT": np.ascontiguousarray(Wq[cols, :].T),
            "wkT": np.ascontiguousarray(Wk[cols, :].T),
            "wvT": np.ascontiguousarray(Wv[cols, :].T),
            "woT": np.ascontiguousarray(Wo[:, cols].T),
            "bq": np.ascontiguousarray(bq[cols]),
            "bk": np.ascontiguousarray(bk[cols]),
        })

    nc = _get_nc()
    global LAST_EXEC_NS, LAST_RESULTS
    res = run_bass_kernel_spmd(nc, in_maps, list(range(N_CORES)),
                               trace=TRACE)
    LAST_EXEC_NS = res.exec_time_ns
    LAST_RESULTS = res

    y = np.empty((B, T, D), dtype=np.float32)
    for b in range(B):
        acc = res.results[2 * b]["y"].astype(np.float32) + \
            res.results[2 * b + 1]["y"].astype(np.float32)
        # exact host-side bias corrections: softmax rows sum to 1, so the
        # v-bias contributes Wo[:, cols] @ bv[cols] per group; plus bo.
        acc += bo[None, :]
        acc += (Wo[:, 0:EL] @ bv[0:EL] + Wo[:, EL:2 * EL] @ bv[EL:2 * EL])[None, :]
        y[b] = acc
    return y


# revision 14
# speedup vs baseline: 1.2494x; 1.1460x over previous
"""Trainium2 Bass kernel for a multi-head attention block.

Problem: y = AttnBlock(x) with B=4, T=2048, D=1024, H=16 heads, head_dim=64.
    q = x@Wq.T+bq, k = x@Wk.T+bk, v = x@Wv.T+bv  (per-head reshape)
    y = softmax(q k^T / 8) v, concat heads, y@Wo.T+bo

Sharding over 8 cores: data-parallel over B (4) x tensor-parallel over head
groups (2 groups of 8 heads).  Each core computes its batch's attention for
its 8 heads plus the partial output projection over its 512 columns of Wo;
the two partials per batch are summed on the host (plus exact host-side
bias corrections for bv/bo).

Device dataflow (per core, all matmuls on TensorE as float32r except the
attention-weight matmul which is bf16):
    phase 1: qT,kT [512,2048] = WqT.T @ xT   (e on partitions -> per-
             partition bias add on DVE), v [2048,520-packed] bf16
    phase 2: per (q-chunk 512, head-pair): S^T chunks [128k,512q] via
             row-tiled K=64 matmul pairs; exp on ScalarE (scale=1/8,
             no max-subtraction: scores ~ N(0,1), exp is safe) -> bf16;
             AV matmul with a fused ones-column producing unnormalized
             O^T [64,512] + sumexp row; normalize via DVE reciprocal +
             DMA partition-broadcast + DVE multiply.
    phase 3: y[t,e] = O^T.T @ WoT accumulated over all 8 heads.
"""

import math

import numpy as np

B, T, D = 4, 2048, 1024
H, HD = 16, 64
EL = 512  # local (per-core) e-width: 8 heads * 64
N_CORES = 8
NPAIR = 4  # head pairs per core
QC = 4  # q chunks of 512
KC = 16  # k chunks of 128
DC = 8  # d chunks of 128
VW = 65  # v columns per head incl. ones column


def build_kernel():
    import concourse.bass as bass
    import concourse.mybir as mybir
    import concourse.tile as tile
    from concourse import bacc

    f32 = mybir.dt.float32
    f32r = mybir.dt.float32r
    bf16 = mybir.dt.bfloat16
    Exp = mybir.ActivationFunctionType.Exp
    Copy = mybir.ActivationFunctionType.Copy

    nc = bacc.Bacc("TRN2", target_bir_lowering=False, debug=False,
                   num_devices=N_CORES)

    xT = nc.dram_tensor("xT", [D, T], bf16, kind="ExternalInput").ap()
    wqT = nc.dram_tensor("wqT", [D, EL], bf16, kind="ExternalInput").ap()
    wkT = nc.dram_tensor("wkT", [D, EL], bf16, kind="ExternalInput").ap()
    wvT = nc.dram_tensor("wvT", [D, EL], bf16, kind="ExternalInput").ap()
    woT = nc.dram_tensor("woT", [EL, D], bf16, kind="ExternalInput").ap()
    bq_d = nc.dram_tensor("bq", [EL], f32, kind="ExternalInput").ap()
    bk_d = nc.dram_tensor("bk", [EL], f32, kind="ExternalInput").ap()
    y_d = nc.dram_tensor("y", [T, D], f32, kind="ExternalOutput").ap()

    with tile.TileContext(nc) as tc:
        with (
            tc.tile_pool(name="p_const", bufs=1) as p_const,
            tc.tile_pool(name="p_qk", bufs=1) as p_qk,
            tc.tile_pool(name="p_v", bufs=1) as p_v,
            tc.tile_pool(name="p_wo", bufs=1) as p_wo,
            tc.tile_pool(name="p_ps_av", bufs=1, space="PSUM") as p_ps_av,
        ):
            # ---- constants: bias chunks [128,1] ----
            bq_t = []
            bk_t = []
            for ec in range(4):
                bqc = p_const.tile([128, 1], f32, name=f"bq{ec}")
                bkc = p_const.tile([128, 1], f32, name=f"bk{ec}")
                nc.sync.dma_start(
                    bqc[:], bq_d[bass.ds(ec * 128, 128)].rearrange(
                        "(p o) -> p o", o=1))
                nc.sync.dma_start(
                    bkc[:], bk_d[bass.ds(ec * 128, 128)].rearrange(
                        "(p o) -> p o", o=1))
                bq_t.append(bqc)
                bk_t.append(bkc)

            # ---- output-proj weights (used late; DMA early, they are small)
            wo_t = []
            for cc in range(4):
                w = p_wo.tile([128, D], bf16, name=f"wo{cc}")
                nc.sync.dma_start(w[:], woT[bass.ds(cc * 128, 128), :])
                wo_t.append(w)

            # ---- qT/kT destination tiles [128, T] (one per head pair) ----
            qT_t = [p_qk.tile([128, T], bf16, name=f"qT{p}") for p in range(NPAIR)]
            kT_t = [p_qk.tile([128, T], bf16, name=f"kT{p}") for p in range(NPAIR)]
            # ---- v tiles: [128, 8*65] bf16 per k-chunk; per head: 64 v cols
            # then a ones column ----
            v_t = [p_v.tile([128, H // 2 * VW], bf16, name=f"v{kc}")
                   for kc in range(KC)]
            for kc in range(KC):
                nc.vector.memset(
                    v_t[kc][:].rearrange("p (h c) -> p h c", c=VW)[:, :, 64:65],
                    1.0)

            # ================= phase 1: projections =================
            with (
                tc.tile_pool(name="p_w", bufs=1) as p_w,
                tc.tile_pool(name="p_x", bufs=1) as p_x,
                tc.tile_pool(name="p_ps_p1", bufs=1, space="PSUM") as p_ps_p1,
            ):
                wq_t, wk_t, wv_t = [], [], []
                for dc in range(DC):
                    for lst, src, nm in ((wq_t, wqT, "q"), (wk_t, wkT, "k"),
                                         (wv_t, wvT, "v")):
                        w = p_w.tile([128, EL], bf16, name=f"w{nm}{dc}")
                        nc.sync.dma_start(
                            w[:], src[bass.ds(dc * 128, 128), :])
                        lst.append(w)

                for tc4 in range(4):  # t-chunks of 512
                    tsl = bass.ds(tc4 * 512, 512)
                    x_t = []
                    for dc in range(DC):
                        xx = p_x.tile([128, 512], bf16,
                                       name=f"x{tc4}_{dc}", tag="x", bufs=12)
                        nc.sync.dma_start(
                            xx[:], xT[bass.ds(dc * 128, 128), tsl])
                        x_t.append(xx)
                    # qT / kT: out [e 128, t 512]
                    for w_list, dst, b_list in ((wq_t, qT_t, bq_t),
                                                (wk_t, kT_t, bk_t)):
                        for ec in range(4):
                            ps = p_ps_p1.tile([128, 512], f32,
                                               name=f"psp{tc4}_{ec}",
                                               tag="p1", bufs=4)
                            for dc in range(DC):
                                nc.tensor.matmul(
                                    ps[:],
                                    w_list[dc][:, bass.ds(ec * 128, 128)],
                                    x_t[dc][:],
                                    start=(dc == 0), stop=(dc == DC - 1))
                            nc.vector.tensor_scalar_add(
                                dst[ec][:, tsl], ps[:], b_list[ec][:])
                    # v: out [t 128, e 512] -> packed bf16 with ones cols
                    for ts in range(4):
                        kc = tc4 * 4 + ts
                        ps = p_ps_p1.tile([128, 512], f32, name=f"psv{kc}", tag="p1", bufs=4)
                        for dc in range(DC):
                            nc.tensor.matmul(
                                ps[:],
                                x_t[dc][:, bass.ds(ts * 128, 128)],
                                wv_t[dc][:],
                                start=(dc == 0), stop=(dc == DC - 1))
                        nc.scalar.activation(
                            v_t[kc][:].rearrange(
                                "p (h c) -> p h c", c=VW)[:, :, 0:64],
                            ps[:].rearrange("p (h c) -> p h c", c=64),
                            Copy)

            # ================= phase 2: attention =================
            # Emission order is tuned to keep the in-order PE stream dense:
            # per (qc, pair) we alternate score-chunk matmuls with the AV
            # matmuls of the previous chunk, and sprinkle the previous
            # q-chunk's output-projection groups between pairs.
            with (
                tc.tile_pool(name="p_es", bufs=1) as p_es,
                tc.tile_pool(name="p_ot", bufs=1) as p_ot,
                tc.tile_pool(name="p_bc", bufs=1) as p_bc,
                tc.tile_pool(name="p_y", bufs=1) as p_y,
                tc.tile_pool(name="p_ps_sc", bufs=1, space="PSUM") as p_ps_sc,
            ):
                oT_t = [p_ot.tile([128, T], bf16, name=f"oT{p}")
                        for p in range(NPAIR)]

                def emit_outproj_group(qc, ts, ecc):
                    t128 = bass.ds(qc * 512 + ts * 128, 128)
                    psy = p_ps_av.tile([128, 512], f32,
                                       name=f"psy{qc}_{ts}_{ecc}",
                                       tag="py", bufs=2)
                    for cc in range(4):
                        nc.tensor.matmul(
                            psy[:],
                            oT_t[cc][:, t128],
                            wo_t[cc][:, bass.ds(ecc * 512, 512)],
                            start=(cc == 0), stop=(cc == 3))
                    yt = p_y.tile([128, 512], f32,
                                  name=f"y{qc}_{ts}_{ecc}", tag="y", bufs=4)
                    nc.vector.tensor_copy(yt[:], psy[:])
                    nc.sync.dma_start(
                        y_d[t128, bass.ds(ecc * 512, 512)], yt[:])

                def emit_av(pair, kc2, es, psA, psB):
                    hA, hB = 2 * pair, 2 * pair + 1
                    for j in range(2):
                        kc = kc2 * 2 + j
                        off = j * 1024
                        nc.tensor.matmul(
                            psA[0:VW, :],
                            v_t[kc][:, bass.ds(hA * VW, VW)],
                            es[:, bass.ds(off, 512)],
                            start=(kc == 0), stop=(kc == KC - 1))
                        nc.tensor.matmul(
                            psB[0:VW, :],
                            v_t[kc][:, bass.ds(hB * VW, VW)],
                            es[:, bass.ds(off + 512, 512)],
                            start=(kc == 0), stop=(kc == KC - 1))

                for qc in range(QC):
                    qsl = bass.ds(qc * 512, 512)
                    st_t = []
                    for pair in range(NPAIR):
                        psA = p_ps_av.tile([128, 512], f32,
                                           name=f"psA{qc}_{pair}", tag="av",
                                           bufs=2)
                        psB = p_ps_av.tile([128, 512], f32,
                                           name=f"psB{qc}_{pair}", tag="av",
                                           bufs=2)
                        es_t = []
                        for kc2 in range(KC // 2):
                            es = p_es.tile([128, 2048], bf16,
                                           name=f"es{qc}_{pair}_{kc2}",
                                           tag="es", bufs=10)
                            for j in range(2):
                                kc = kc2 * 2 + j
                                ksl = bass.ds(kc * 128, 128)
                                sc = p_ps_sc.tile([128, 1024], f32,
                                                  name=f"sc{qc}_{pair}_{kc}",
                                                  tag="sc", bufs=2)
                                nc.tensor.matmul(
                                    sc[:, 0:512],
                                    kT_t[pair][0:64, ksl],
                                    qT_t[pair][0:64, qsl],
                                    start=True, stop=True)
                                nc.tensor.matmul(
                                    sc[:, 512:1024],
                                    kT_t[pair][64:128, ksl],
                                    qT_t[pair][64:128, qsl],
                                    start=True, stop=True)
                                nc.scalar.activation(
                                    es[:, bass.ds(j * 1024, 1024)], sc[:],
                                    Exp, scale=1.0 / math.sqrt(HD))
                            es_t.append(es)
                            if kc2 >= 1:
                                emit_av(pair, kc2 - 1, es_t[kc2 - 1], psA, psB)
                            if qc > 0 and pair >= 1 and kc2 in (1, 5):
                                g6 = (pair - 1) * 2 + (0 if kc2 == 1 else 1)
                                emit_outproj_group(qc - 1, g6 // 2, g6 % 2)
                        emit_av(pair, 7, es_t[7], psA, psB)
                        # copy accumulators to SBUF so the PSUM banks recycle
                        # immediately; normalization happens once per q-chunk
                        stA = p_bc.tile([VW, 512], f32,
                                        name=f"stA{qc}_{pair}", tag="st",
                                        bufs=10)
                        stB = p_bc.tile([VW, 512], f32,
                                        name=f"stB{qc}_{pair}", tag="st",
                                        bufs=10)
                        nc.vector.tensor_copy(stA[:], psA[0:VW, :])
                        nc.vector.tensor_copy(stB[:], psB[0:VW, :])
                        st_t.append((pair, stA, stB))

                    # --- deferred normalize for the whole q-chunk: one
                    # 8-lane reciprocal instead of eight 1-lane ones ---
                    g = p_bc.tile([8, 512], f32, name=f"g{qc}", tag="g",
                                  bufs=2)
                    rg = p_bc.tile([8, 512], f32, name=f"rg{qc}", tag="rg",
                                   bufs=2)
                    for i, (pair, stA, stB) in enumerate(st_t):
                        nc.sync.dma_start(g[2 * i:2 * i + 1, :],
                                          stA[64:65, :])
                        nc.sync.dma_start(g[2 * i + 1:2 * i + 2, :],
                                          stB[64:65, :])
                    nc.vector.reciprocal(rg[:], g[:])
                    for i, (pair, stA, stB) in enumerate(st_t):
                        bcA = p_bc.tile([64, 512], f32,
                                        name=f"bcA{qc}_{pair}", tag="bc",
                                        bufs=6)
                        bcB = p_bc.tile([64, 512], f32,
                                        name=f"bcB{qc}_{pair}", tag="bc",
                                        bufs=6)
                        nc.sync.dma_start(
                            bcA[:],
                            rg[2 * i:2 * i + 1, :].rearrange(
                                "p (o f) -> p o f", o=1).to_broadcast(
                                    (1, 64, 512)))
                        nc.sync.dma_start(
                            bcB[:],
                            rg[2 * i + 1:2 * i + 2, :].rearrange(
                                "p (o f) -> p o f", o=1).to_broadcast(
                                    (1, 64, 512)))
                        nc.vector.tensor_mul(
                            oT_t[pair][0:64, qsl], stA[0:64, :], bcA[:])
                        # head B: normalize at partitions 0:64, then DMA-hop
                        # to partitions 64:128 of the O^T tile
                        tmp = p_bc.tile([64, 512], bf16,
                                        name=f"tmp{qc}_{pair}", tag="tmp",
                                        bufs=4)
                        nc.vector.tensor_mul(tmp[:], stB[0:64, :], bcB[:])
                        nc.sync.dma_start(oT_t[pair][64:128, qsl], tmp[:])
                    if qc > 0:
                        emit_outproj_group(qc - 1, 3, 0)
                        emit_outproj_group(qc - 1, 3, 1)

                # tail: output projection for the last q-chunk
                for ts in range(4):
                    for ecc in range(2):
                        emit_outproj_group(QC - 1, ts, ecc)

    nc.compile()
    return nc


_NC_CACHE = None
TRACE = False
LAST_EXEC_NS = None
LAST_RESULTS = None


def _get_nc():
    global _NC_CACHE
    if _NC_CACHE is None:
        _NC_CACHE = build_kernel()
    return _NC_CACHE


def kernel(x, Wq, bq, Wk, bk, Wv, bv, Wo, bo):
    from concourse.bass_utils import run_bass_kernel_spmd

    x = np.ascontiguousarray(np.asarray(x, dtype=np.float32))
    Wq = np.asarray(Wq, dtype=np.float32)
    Wk = np.asarray(Wk, dtype=np.float32)
    Wv = np.asarray(Wv, dtype=np.float32)
    Wo = np.asarray(Wo, dtype=np.float32)
    bq = np.asarray(bq, dtype=np.float32)
    bk = np.asarray(bk, dtype=np.float32)
    bv = np.asarray(bv, dtype=np.float32)
    bo = np.asarray(bo, dtype=np.float32)

    import ml_dtypes

    bf16 = ml_dtypes.bfloat16
    in_maps = []
    for core in range(N_CORES):
        b = core // 2
        g = core % 2
        cols = slice(g * EL, (g + 1) * EL)
        in_maps.append({
            "xT": np.ascontiguousarray(x[b].T).astype(bf16),
            "wqT": np.ascontiguousarray(Wq[cols, :].T).astype(bf16),
            "wkT": np.ascontiguousarray(Wk[cols, :].T).astype(bf16),
            "wvT": np.ascontiguousarray(Wv[cols, :].T).astype(bf16),
            "woT": np.ascontiguousarray(Wo[:, cols].T).astype(bf16),
            "bq": np.ascontiguousarray(bq[cols]),
            "bk": np.ascontiguousarray(bk[cols]),
        })

    nc = _get_nc()
    global LAST_EXEC_NS, LAST_RESULTS
    res = run_bass_kernel_spmd(nc, in_maps, list(range(N_CORES)),
                               trace=TRACE)
    LAST_EXEC_NS = res.exec_time_ns
    LAST_RESULTS = res

    y = np.empty((B, T, D), dtype=np.float32)
    for b in range(B):
        acc = res.results[2 * b]["y"].astype(np.float32) + \
            res.results[2 * b + 1]["y"].astype(np.float32)
        # exact host-side bias corrections: softmax rows sum to 1, so the
        # v-bias contributes Wo[:, cols] @ bv[cols] per group; plus bo.
        acc += bo[None, :]
        acc += (Wo[:, 0:EL] @ bv[0:EL] + Wo[:, EL:2 * EL] @ bv[EL:2 * EL])[None, :]
        y[b] = acc
    return y
